# revision 12
# baseline (speedup 1.0000x reference)
"""DeepSpeedMLP (pre-LN fp32 path) on 8 Trainium2 NeuronCores.

Sharding: data-parallel over tokens (8192 tokens -> 1024/core); every core
streams the full inter_w/output_w (converted to bf16 on host) from HBM once.

Per-core pipeline (v2):
  stage 1: res = input+residual+bias (bias host-fused into r); LN stats;
           ln_bf = (res-mu)*rstd cast to bf16; PE 128x128 transposes ->
           lnT[h, t] with gamma/beta applied during the PSUM->SBUF evict.
           res is stored to o_d (seeds the final residual add).
  stage 2: for each 2048-wide I-block b:
           GEMM1  psum[i, t] += W1[k, iblk].T @ lnT[k, t]   (W1 stationary,
                  tokens moving; output lands already transposed)
           gelu:  inter[i, t] = Gelu(psum + b1)             (ACT, bf16)
           GEMM2  psum[t, h] += inter[i, t].T @ W2[iblk, h]
           evict: out_acc (SBUF bf16, seeded with res in stage 1) += psum
           via DVE; the last block adds b2 (K=1 ones-matmul into psum) and
           stores out_acc + psum to o_d in f32 (sole writer of o_d).

All matmuls are bf16 (fp32 PSUM accumulation): ~1e-3 relative error.
Weight DMAs are 1MB on the SP HWDGE queue; activations on the ACT HWDGE
queue; output accumulation on the SWDGE (gpsimd) queue.
"""
import sys
if '/opt/trn_rl_repo' not in sys.path:
    sys.path.insert(0, '/opt/trn_rl_repo')

import numpy as np
import concourse.bass as bass
import concourse.mybir as mybir
import concourse.tile as tile
from concourse import bacc
from concourse.bass_utils import run_bass_kernel_spmd

dt = mybir.dt
AF = mybir.ActivationFunctionType
ALU = mybir.AluOpType

N_CORES = 8
B, S, HIDDEN, INTER = 4, 2048, 4096, 16384
TOK = B * S
T = TOK // N_CORES       # tokens per core
IBLK = 2048              # I-block width
EPS = 1e-5

BF = dt.bfloat16


def _build_nc(H, I, T, IBLK):
    KS = H // 128       # h k-slabs
    TT = T // 128       # token tiles
    NB = I // IBLK      # I blocks
    IS = IBLK // 128    # 128-row i-slabs per block
    NG = IBLK // 512    # 512-wide i groups per block (4 i-tiles each)
    TC = T // 512       # 512-token chunks (GEMM1 moving N)
    HC = H // 512       # 512-wide h-chunks (GEMM2 moving N)
    WKC = min(4, KS)    # k-slabs per w1 DMA chunk
    KC = KS // WKC      # w1 DMA chunks per group
    W2C = min(8, IS)    # i-slabs per w2 DMA chunk
    SW = min(H, 2048)   # stage-1 strip width
    NS = H // SW

    nc = bacc.Bacc(None, target_bir_lowering=False)
    P = nc.declare_dram_parameter
    x_d = P("x", [T, H], dt.float32, isOutput=False)
    r_d = P("r", [T, H], dt.float32, isOutput=False)
    g_d = P("gamma_t", [128, KS], dt.float32, isOutput=False)
    be_d = P("beta_t", [128, KS], dt.float32, isOutput=False)
    w1_d = P("w1", [H, I], BF, isOutput=False)
    b1_d = P("b1_t", [128, I // 128], dt.float32, isOutput=False)
    w2_d = P("w2", [I, H], BF, isOutput=False)
    b2_d = P("b2", [1, H], BF, isOutput=False)
    ones_d = P("ones", [1, 128], BF, isOutput=False)
    id_d = P("ident", [128, 128], BF, isOutput=False)
    o_d = P("out", [T, H], dt.float32, isOutput=True)

    w1_v = w1_d[:, :].rearrange("(k p) i -> p k i", p=128)   # [128, KS, I]
    w2_v = w2_d[:, :].rearrange("(s p) h -> p s h", p=128)   # [128, I/128, H]

    with tile.TileContext(nc) as tc:
        with (
            tc.tile_pool(name="const", bufs=1) as constp,
            tc.tile_pool(name="lnT", bufs=1) as lnTp,
            tc.tile_pool(name="psum", bufs=8, space="PSUM") as psum,
        ):
            ident = constp.tile([128, 128], BF)
            nc.sync.dma_start(out=ident[:], in_=id_d[:])
            g_sb = constp.tile([128, KS], dt.float32)
            nc.sync.dma_start(out=g_sb[:], in_=g_d[:])
            be_sb = constp.tile([128, KS], dt.float32)
            nc.sync.dma_start(out=be_sb[:], in_=be_d[:])
            b1_sb = constp.tile([128, I // 128], dt.float32)
            nc.sync.dma_start(out=b1_sb[:], in_=b1_d[:])
            ones = constp.tile([1, 128], BF)
            nc.sync.dma_start(out=ones[:], in_=ones_d[:])

            lnT = lnTp.tile([128, KS, T], BF)
            out_acc = lnTp.tile([128, TT, H], BF, name="out_acc")

            # ---- Stage 1: residual add + LN + transpose ----
            with (
                tc.tile_pool(name="s1in", bufs=2) as inp,
                tc.tile_pool(name="s1res", bufs=1) as resp,
                tc.tile_pool(name="s1ln", bufs=2) as lnbp,
                tc.tile_pool(name="s1st", bufs=2) as stp,
            ):
                for t in range(TT):
                    tr = slice(t * 128, (t + 1) * 128)
                    res = resp.tile([128, H], dt.float32, name="res")
                    for s in range(NS):
                        cs = slice(s * SW, (s + 1) * SW)
                        xs = inp.tile([128, SW], dt.float32, name="xt")
                        rs = inp.tile([128, SW], dt.float32, name="xt")
                        nc.scalar.dma_start(out=xs[:], in_=x_d[tr, cs])
                        nc.scalar.dma_start(out=rs[:], in_=r_d[tr, cs])
                        nc.vector.tensor_add(res[:, cs], xs[:], rs[:])
                    nc.vector.tensor_copy(out_acc[:, t, :], res[:])

                    s1 = stp.tile([128, 1], dt.float32, name="s1")
                    nc.vector.tensor_reduce(s1[:], res[:], mybir.AxisListType.X, ALU.add)
                    s2 = stp.tile([128, 1], dt.float32, name="s2")
                    for s in range(NS):
                        cs = slice(s * SW, (s + 1) * SW)
                        junk = inp.tile([128, SW], dt.float32, name="xt")
                        s2p = stp.tile([128, 1], dt.float32, name="s2p")
                        nc.scalar.activation(junk[:], res[:, cs], AF.Square,
                                             accum_out=s2p[:])
                        if s == 0:
                            nc.vector.tensor_copy(s2[:], s2p[:])
                        else:
                            nc.vector.tensor_add(s2[:], s2[:], s2p[:])
                    mu = stp.tile([128, 1], dt.float32, name="mu")
                    nc.vector.tensor_scalar_mul(mu[:], s1[:], 1.0 / H)
                    mu2 = stp.tile([128, 1], dt.float32, name="mu2")
                    nc.vector.tensor_mul(mu2[:], mu[:], mu[:])
                    var = stp.tile([128, 1], dt.float32, name="var")
                    nc.vector.tensor_scalar(var[:], s2[:], 1.0 / H, float(EPS),
                                            ALU.mult, ALU.add)
                    nc.vector.tensor_sub(var[:], var[:], mu2[:])
                    sd = stp.tile([128, 1], dt.float32, name="sd")
                    nc.scalar.activation(sd[:], var[:], AF.Sqrt)
                    rstd = stp.tile([128, 1], dt.float32, name="rstd")
                    nc.vector.reciprocal(rstd[:], sd[:])
                    nmr = stp.tile([128, 1], dt.float32, name="nmr")
                    nc.vector.tensor_mul(nmr[:], mu[:], rstd[:])
                    nc.vector.tensor_scalar_mul(nmr[:], nmr[:], -1.0)

                    lnb = lnbp.tile([128, H], BF, name="lnb")
                    for s in range(NS):
                        cs = slice(s * SW, (s + 1) * SW)
                        nc.scalar.activation(lnb[:, cs], res[:, cs], AF.Identity,
                                             bias=nmr[:], scale=rstd[:])
                    for q in range(KS // 4):
                        pt = psum.tile([128, 4, 128], BF, name="ps")
                        for j in range(4):
                            nc.tensor.transpose(
                                pt[:, j, :],
                                lnb[:, (q * 4 + j) * 128:(q * 4 + j + 1) * 128],
                                ident[:])
                        for j in range(4):
                            k = q * 4 + j
                            nc.vector.tensor_scalar(
                                lnT[:, k, t * 128:(t + 1) * 128],
                                pt[:, j, :],
                                g_sb[:, k:k + 1], be_sb[:, k:k + 1],
                                ALU.mult, ALU.add)

            # ---- Stage 2: per I-block GEMM1 -> gelu -> GEMM2 -> accum ----
            with (
                tc.tile_pool(name="interp", bufs=1) as interp,
                tc.tile_pool(name="w1p", bufs=3) as w1p,
                tc.tile_pool(name="w2p", bufs=2) as w2p,
                tc.tile_pool(name="stage", bufs=3) as stgp,
                tc.tile_pool(name="b2p", bufs=2) as b2p,
            ):
                for b in range(NB):
                    last = (b == NB - 1)
                    inter = interp.tile([128, IS, T], BF, name="inter")
                    # GEMM1: psum[i, t] += W1.T @ lnT over all k
                    for g in range(NG):
                        pA = [psum.tile([128, 512], dt.float32, name="ps")
                              for _ in range(4 * TC)]
                        i0 = b * IBLK + g * 512
                        for kc in range(KC):
                            w1t = w1p.tile([128, WKC, 512], BF, name="w1t")
                            nc.sync.dma_start(
                                out=w1t[:],
                                in_=w1_v[:, kc * WKC:(kc + 1) * WKC, i0:i0 + 512])
                            for j in range(WKC):
                                k = kc * WKC + j
                                for it in range(4):
                                    for c in range(TC):
                                        nc.tensor.matmul(
                                            pA[it * TC + c][:],
                                            w1t[:, j, it * 128:(it + 1) * 128],
                                            lnT[:, k, c * 512:(c + 1) * 512],
                                            start=(k == 0), stop=(k == KS - 1))
                        for it in range(4):
                            slab = g * 4 + it
                            gs = b * IS + slab
                            for c in range(TC):
                                nc.scalar.activation(
                                    inter[:, slab, c * 512:(c + 1) * 512],
                                    pA[it * TC + c][:], AF.Gelu_apprx_tanh,
                                    bias=b1_sb[:, gs:gs + 1])
                    # GEMM2: psum[t, h] += inter.T @ W2 over block's i-slabs
                    for h in range(HC):
                        hs = slice(h * 512, (h + 1) * 512)
                        w2t = []
                        for sc in range(IS // W2C):
                            w2c = w2p.tile([128, W2C, 512], BF, name="w2t")
                            nc.sync.dma_start(
                                out=w2c[:],
                                in_=w2_v[:, b * IS + sc * W2C:
                                         b * IS + (sc + 1) * W2C, hs])
                            w2t.append(w2c)
                        if last:
                            b2s = b2p.tile([1, 512], BF, name="b2s")
                            nc.sync.dma_start(out=b2s[:], in_=b2_d[:, hs])
                        for t in range(TT):
                            tr = slice(t * 128, (t + 1) * 128)
                            pB = psum.tile([128, 512], dt.float32, name="ps")
                            for s in range(IS):
                                nc.tensor.matmul(
                                    pB[:],
                                    inter[:, s, t * 128:(t + 1) * 128],
                                    w2t[s // W2C][:, s % W2C, :],
                                    start=(s == 0),
                                    stop=(s == IS - 1) and not last)
                            acc = out_acc[:, t, hs]
                            if last:
                                nc.tensor.matmul(pB[:], ones[:], b2s[:],
                                                 start=False, stop=True)
                                stg = stgp.tile([128, 512], dt.float32,
                                                name="stg")
                                nc.vector.tensor_add(stg[:], pB[:], acc)
                                nc.scalar.dma_start(out=o_d[tr, hs], in_=stg[:])
                            else:
                                nc.vector.tensor_add(acc, acc, pB[:])
    nc.compile()
    return nc


_NC_CACHE = None
_last_maps = None


def _get_nc():
    global _NC_CACHE
    if _NC_CACHE is None:
        _NC_CACHE = _build_nc(HIDDEN, INTER, T, IBLK)
    return _NC_CACHE


def build_maps(inputs):
    H, I = HIDDEN, INTER
    KS = H // 128
    bf = mybir.dt.np(BF)

    x = np.ascontiguousarray(np.asarray(inputs['input'], np.float32).reshape(TOK, H))
    r2 = np.asarray(inputs['residual'], np.float32).reshape(TOK, H) + \
        np.asarray(inputs['bias'], np.float32)[None, :]
    gamma_t = np.ascontiguousarray(
        np.asarray(inputs['attn_nw'], np.float32).reshape(KS, 128).T)
    beta_t = np.ascontiguousarray(
        np.asarray(inputs['attn_nb'], np.float32).reshape(KS, 128).T)
    b1_t = np.ascontiguousarray(
        np.asarray(inputs['inter_b'], np.float32).reshape(I // 128, 128).T)
    b2 = np.ascontiguousarray(
        np.asarray(inputs['output_b'], np.float32)[None, :]).astype(bf)
    w1 = np.ascontiguousarray(np.asarray(inputs['inter_w'], np.float32)).astype(bf)
    w2 = np.ascontiguousarray(np.asarray(inputs['output_w'], np.float32)).astype(bf)
    ident = np.eye(128, dtype=np.float32).astype(bf)
    ones = np.ones((1, 128), np.float32).astype(bf)

    maps = []
    for c in range(N_CORES):
        sl = slice(c * T, (c + 1) * T)
        maps.append({
            'x': x[sl], 'r': np.ascontiguousarray(r2[sl]).astype(np.float32),
            'gamma_t': gamma_t, 'beta_t': beta_t,
            'w1': w1, 'b1_t': b1_t, 'w2': w2, 'b2': b2,
            'ones': ones, 'ident': ident,
        })
    return maps


def kernel(input, residual, residual_norm, bias, attn_nw, attn_nb,
           inter_w, inter_b, output_w, output_b, **kwargs):
    global _last_maps
    nc = _get_nc()
    maps = build_maps({
        'input': input, 'residual': residual, 'bias': bias,
        'attn_nw': attn_nw, 'attn_nb': attn_nb, 'inter_w': inter_w,
        'inter_b': inter_b, 'output_w': output_w, 'output_b': output_b,
    })
    _last_maps = maps
    res = run_bass_kernel_spmd(nc, maps, list(range(N_CORES)))
    out = np.concatenate([res.results[c]['out'] for c in range(N_CORES)], 0)
    return out.reshape(B, S, HIDDEN).astype(np.float32)


# revision 13
# speedup vs baseline: 1.0100x; 1.0100x over previous
"""DeepSpeedMLP (pre-LN fp32 path) on 8 Trainium2 NeuronCores.

Sharding: data-parallel over tokens (8192 tokens -> 1024/core); every core
streams the full inter_w/output_w (converted to bf16 on host) from HBM once.

Per-core pipeline (v2):
  stage 1: res = input+residual+bias (bias host-fused into r); LN stats;
           ln_bf = (res-mu)*rstd cast to bf16; PE 128x128 transposes ->
           lnT[h, t] with gamma/beta applied during the PSUM->SBUF evict.
           res is stored to o_d (seeds the final residual add).
  stage 2: for each 2048-wide I-block b:
           GEMM1  psum[i, t] += W1[k, iblk].T @ lnT[k, t]   (W1 stationary,
                  tokens moving; output lands already transposed)
           gelu:  inter[i, t] = Gelu(psum + b1)             (ACT, bf16)
           GEMM2  psum[t, h] += inter[i, t].T @ W2[iblk, h]
           evict: out_acc (SBUF bf16, seeded with res in stage 1) += psum
           via DVE; the last block adds b2 (K=1 ones-matmul into psum) and
           stores out_acc + psum to o_d in f32 (sole writer of o_d).

All matmuls are bf16 (fp32 PSUM accumulation): ~1e-3 relative error.
Weight DMAs are 1MB on the SP HWDGE queue; activations on the ACT HWDGE
queue; output accumulation on the SWDGE (gpsimd) queue.
"""
import sys
if '/opt/trn_rl_repo' not in sys.path:
    sys.path.insert(0, '/opt/trn_rl_repo')

import numpy as np
import concourse.bass as bass
import concourse.mybir as mybir
import concourse.tile as tile
from concourse import bacc
from concourse.bass_utils import run_bass_kernel_spmd

dt = mybir.dt
AF = mybir.ActivationFunctionType
ALU = mybir.AluOpType

N_CORES = 8
B, S, HIDDEN, INTER = 4, 2048, 4096, 16384
TOK = B * S
T = TOK // N_CORES       # tokens per core
IBLK = 2048              # I-block width
EPS = 1e-5

BF = dt.bfloat16


def _build_nc(H, I, T, IBLK):
    KS = H // 128       # h k-slabs
    TT = T // 128       # token tiles
    NB = I // IBLK      # I blocks
    IS = IBLK // 128    # 128-row i-slabs per block
    NG = IBLK // 512    # 512-wide i groups per block (4 i-tiles each)
    TC = T // 512       # 512-token chunks (GEMM1 moving N)
    HC = H // 512       # 512-wide h-chunks (GEMM2 moving N)
    WKC = min(4, KS)    # k-slabs per w1 DMA chunk
    KC = KS // WKC      # w1 DMA chunks per group
    W2C = min(8, IS)    # i-slabs per w2 DMA chunk
    SW = min(H, 2048)   # stage-1 strip width
    NS = H // SW

    nc = bacc.Bacc(None, target_bir_lowering=False)
    P = nc.declare_dram_parameter
    x_d = P("x", [T, H], dt.float32, isOutput=False)
    r_d = P("r", [T, H], dt.float32, isOutput=False)
    g_d = P("gamma_t", [128, KS], dt.float32, isOutput=False)
    be_d = P("beta_t", [128, KS], dt.float32, isOutput=False)
    w1_d = P("w1", [H, I], BF, isOutput=False)
    b1_d = P("b1_t", [128, I // 128], dt.float32, isOutput=False)
    w2_d = P("w2", [I, H], BF, isOutput=False)
    b2_d = P("b2", [1, H], BF, isOutput=False)
    ones_d = P("ones", [1, 128], BF, isOutput=False)
    id_d = P("ident", [128, 128], BF, isOutput=False)
    o_d = P("out", [T, H], dt.float32, isOutput=True)

    w1_v = w1_d[:, :].rearrange("(k p) i -> p k i", p=128)   # [128, KS, I]
    w2_v = w2_d[:, :].rearrange("(s p) h -> p s h", p=128)   # [128, I/128, H]

    with tile.TileContext(nc) as tc:
        with (
            tc.tile_pool(name="const", bufs=1) as constp,
            tc.tile_pool(name="lnT", bufs=1) as lnTp,
            tc.tile_pool(name="psum", bufs=8, space="PSUM") as psum,
        ):
            ident = constp.tile([128, 128], BF)
            nc.sync.dma_start(out=ident[:], in_=id_d[:])
            g_sb = constp.tile([128, KS], dt.float32)
            nc.sync.dma_start(out=g_sb[:], in_=g_d[:])
            be_sb = constp.tile([128, KS], dt.float32)
            nc.sync.dma_start(out=be_sb[:], in_=be_d[:])
            b1_sb = constp.tile([128, I // 128], dt.float32)
            nc.sync.dma_start(out=b1_sb[:], in_=b1_d[:])
            ones = constp.tile([1, 128], BF)
            nc.sync.dma_start(out=ones[:], in_=ones_d[:])

            lnT = lnTp.tile([128, KS, T], BF)
            out_acc = lnTp.tile([128, TT, H], BF, name="out_acc")

            # ---- Stage 1: residual add + LN + transpose ----
            with (
                tc.tile_pool(name="s1in", bufs=2) as inp,
                tc.tile_pool(name="s1res", bufs=1) as resp,
                tc.tile_pool(name="s1ln", bufs=2) as lnbp,
                tc.tile_pool(name="s1st", bufs=2) as stp,
            ):
                for t in range(TT):
                    tr = slice(t * 128, (t + 1) * 128)
                    res = resp.tile([128, H], dt.float32, name="res")
                    for s in range(NS):
                        cs = slice(s * SW, (s + 1) * SW)
                        xs = inp.tile([128, SW], dt.float32, name="xt")
                        rs = inp.tile([128, SW], dt.float32, name="xt")
                        nc.scalar.dma_start(out=xs[:], in_=x_d[tr, cs])
                        nc.scalar.dma_start(out=rs[:], in_=r_d[tr, cs])
                        nc.vector.tensor_add(res[:, cs], xs[:], rs[:])
                    nc.vector.tensor_copy(out_acc[:, t, :], res[:])

                    s1 = stp.tile([128, 1], dt.float32, name="s1")
                    nc.vector.tensor_reduce(s1[:], res[:], mybir.AxisListType.X, ALU.add)
                    s2 = stp.tile([128, 1], dt.float32, name="s2")
                    for s in range(NS):
                        cs = slice(s * SW, (s + 1) * SW)
                        junk = inp.tile([128, SW], dt.float32, name="xt")
                        s2p = stp.tile([128, 1], dt.float32, name="s2p")
                        nc.scalar.activation(junk[:], res[:, cs], AF.Square,
                                             accum_out=s2p[:])
                        if s == 0:
                            nc.vector.tensor_copy(s2[:], s2p[:])
                        else:
                            nc.vector.tensor_add(s2[:], s2[:], s2p[:])
                    mu = stp.tile([128, 1], dt.float32, name="mu")
                    nc.vector.tensor_scalar_mul(mu[:], s1[:], 1.0 / H)
                    mu2 = stp.tile([128, 1], dt.float32, name="mu2")
                    nc.vector.tensor_mul(mu2[:], mu[:], mu[:])
                    var = stp.tile([128, 1], dt.float32, name="var")
                    nc.vector.tensor_scalar(var[:], s2[:], 1.0 / H, float(EPS),
                                            ALU.mult, ALU.add)
                    nc.vector.tensor_sub(var[:], var[:], mu2[:])
                    sd = stp.tile([128, 1], dt.float32, name="sd")
                    nc.scalar.activation(sd[:], var[:], AF.Sqrt)
                    rstd = stp.tile([128, 1], dt.float32, name="rstd")
                    nc.vector.reciprocal(rstd[:], sd[:])
                    nmr = stp.tile([128, 1], dt.float32, name="nmr")
                    nc.vector.tensor_mul(nmr[:], mu[:], rstd[:])
                    nc.vector.tensor_scalar_mul(nmr[:], nmr[:], -1.0)

                    lnb = lnbp.tile([128, H], BF, name="lnb")
                    for s in range(NS):
                        cs = slice(s * SW, (s + 1) * SW)
                        nc.scalar.activation(lnb[:, cs], res[:, cs], AF.Identity,
                                             bias=nmr[:], scale=rstd[:])
                    for q in range(KS // 4):
                        pt = psum.tile([128, 4, 128], BF, name="ps")
                        for j in range(4):
                            nc.tensor.transpose(
                                pt[:, j, :],
                                lnb[:, (q * 4 + j) * 128:(q * 4 + j + 1) * 128],
                                ident[:])
                        for j in range(4):
                            k = q * 4 + j
                            nc.vector.tensor_scalar(
                                lnT[:, k, t * 128:(t + 1) * 128],
                                pt[:, j, :],
                                g_sb[:, k:k + 1], be_sb[:, k:k + 1],
                                ALU.mult, ALU.add)

            # ---- Stage 2: per I-block GEMM1 -> gelu -> GEMM2 -> accum ----
            with (
                tc.tile_pool(name="interp", bufs=1) as interp,
                tc.tile_pool(name="w1p", bufs=3) as w1p,
                tc.tile_pool(name="w2p", bufs=2) as w2p,
                tc.tile_pool(name="stage", bufs=3) as stgp,
                tc.tile_pool(name="b2p", bufs=2) as b2p,
            ):
                for b in range(NB):
                    last = (b == NB - 1)
                    inter = interp.tile([128, IS, T], BF, name="inter")
                    # GEMM1: psum[i, t] += W1.T @ lnT over all k
                    for g in range(NG):
                        pA = [psum.tile([128, 512], dt.float32, name="ps")
                              for _ in range(4 * TC)]
                        i0 = b * IBLK + g * 512
                        for kc in range(KC):
                            w1t = w1p.tile([128, WKC, 512], BF, name="w1t")
                            nc.sync.dma_start(
                                out=w1t[:],
                                in_=w1_v[:, kc * WKC:(kc + 1) * WKC, i0:i0 + 512])
                            for j in range(WKC):
                                k = kc * WKC + j
                                for it in range(4):
                                    for c in range(TC):
                                        nc.tensor.matmul(
                                            pA[it * TC + c][:],
                                            w1t[:, j, it * 128:(it + 1) * 128],
                                            lnT[:, k, c * 512:(c + 1) * 512],
                                            start=(k == 0), stop=(k == KS - 1))
                        for it in range(4):
                            slab = g * 4 + it
                            gs = b * IS + slab
                            for c in range(TC):
                                nc.scalar.activation(
                                    inter[:, slab, c * 512:(c + 1) * 512],
                                    pA[it * TC + c][:], AF.Gelu_apprx_tanh,
                                    bias=b1_sb[:, gs:gs + 1])
                    # GEMM2: psum[t, h] += inter.T @ W2 over block's i-slabs
                    for h in range(HC):
                        hs = slice(h * 512, (h + 1) * 512)
                        w2t = []
                        for sc in range(IS // W2C):
                            w2c = w2p.tile([128, W2C, 512], BF, name="w2t")
                            nc.sync.dma_start(
                                out=w2c[:],
                                in_=w2_v[:, b * IS + sc * W2C:
                                         b * IS + (sc + 1) * W2C, hs])
                            w2t.append(w2c)
                        if last:
                            b2s = b2p.tile([1, 512], BF, name="b2s")
                            nc.sync.dma_start(out=b2s[:], in_=b2_d[:, hs])
                        for t in range(TT):
                            tr = slice(t * 128, (t + 1) * 128)
                            pB = psum.tile([128, 512], dt.float32, name="ps")
                            for s in range(IS):
                                nc.tensor.matmul(
                                    pB[:],
                                    inter[:, s, t * 128:(t + 1) * 128],
                                    w2t[s // W2C][:, s % W2C, :],
                                    start=(s == 0),
                                    stop=(s == IS - 1) and not last)
                            acc = out_acc[:, t, hs]
                            if last:
                                nc.tensor.matmul(pB[:], ones[:], b2s[:],
                                                 start=False, stop=True)
                                stg = stgp.tile([128, 512], dt.float32,
                                                name="stg")
                                nc.vector.tensor_add(stg[:], pB[:], acc)
                                nc.scalar.dma_start(out=o_d[tr, hs], in_=stg[:])
                            else:
                                nc.vector.tensor_add(acc, acc, pB[:])
    import os
    if not os.environ.get('BASS_SKIP_COMPILE'):
        nc.compile()
    return nc


_NC_CACHE = None
_last_maps = None


def _get_nc():
    global _NC_CACHE
    if _NC_CACHE is None:
        _NC_CACHE = _build_nc(HIDDEN, INTER, T, IBLK)
    return _NC_CACHE


def build_maps(inputs):
    H, I = HIDDEN, INTER
    KS = H // 128
    bf = mybir.dt.np(BF)

    x = np.ascontiguousarray(np.asarray(inputs['input'], np.float32).reshape(TOK, H))
    r2 = np.asarray(inputs['residual'], np.float32).reshape(TOK, H) + \
        np.asarray(inputs['bias'], np.float32)[None, :]
    gamma_t = np.ascontiguousarray(
        np.asarray(inputs['attn_nw'], np.float32).reshape(KS, 128).T)
    beta_t = np.ascontiguousarray(
        np.asarray(inputs['attn_nb'], np.float32).reshape(KS, 128).T)
    b1_t = np.ascontiguousarray(
        np.asarray(inputs['inter_b'], np.float32).reshape(I // 128, 128).T)
    b2 = np.ascontiguousarray(
        np.asarray(inputs['output_b'], np.float32)[None, :]).astype(bf)
    w1 = np.ascontiguousarray(np.asarray(inputs['inter_w'], np.float32)).astype(bf)
    w2 = np.ascontiguousarray(np.asarray(inputs['output_w'], np.float32)).astype(bf)
    ident = np.eye(128, dtype=np.float32).astype(bf)
    ones = np.ones((1, 128), np.float32).astype(bf)

    maps = []
    for c in range(N_CORES):
        sl = slice(c * T, (c + 1) * T)
        maps.append({
            'x': x[sl], 'r': np.ascontiguousarray(r2[sl]).astype(np.float32),
            'gamma_t': gamma_t, 'beta_t': beta_t,
            'w1': w1, 'b1_t': b1_t, 'w2': w2, 'b2': b2,
            'ones': ones, 'ident': ident,
        })
    return maps


def kernel(input, residual, residual_norm, bias, attn_nw, attn_nb,
           inter_w, inter_b, output_w, output_b, **kwargs):
    global _last_maps
    nc = _get_nc()
    maps = build_maps({
        'input': input, 'residual': residual, 'bias': bias,
        'attn_nw': attn_nw, 'attn_nb': attn_nb, 'inter_w': inter_w,
        'inter_b': inter_b, 'output_w': output_w, 'output_b': output_b,
    })
    _last_maps = maps
    res = run_bass_kernel_spmd(nc, maps, list(range(N_CORES)))
    out = np.concatenate([res.results[c]['out'] for c in range(N_CORES)], 0)
    return out.reshape(B, S, HIDDEN).astype(np.float32)
